# revision 4
# baseline (speedup 1.0000x reference)
"""Causal self-attention (B=4, T=2048, D=1024, H=16) on 8 trn2 NeuronCores.

Sharding: core c handles batch b=c//2 and head-group hg=c%2 (8 of 16 heads).
W_q/W_k/W_v are column-sharded per head-group (host-side). After attention,
each pair of cores AllGathers the transposed attention output (channels) and
computes a disjoint half of the output channels of the O-projection
(W_o.T column-sharded per rank parity), so the device program is identical
on every core; the host concatenates the halves.

All matmuls run in bf16 with fp32 PSUM accumulation. Softmax is computed
without max-subtraction (scores are O(1) here; exp is safe in fp32/bf16),
with the denominator obtained from an extra ones-column appended to V.
"""

import os
import sys

for _p in ("/opt/trn_rl_repo", "/root/.axon_site/_ro/trn_rl_repo"):
    if os.path.isdir(_p) and _p not in sys.path:
        sys.path.insert(0, _p)

import ml_dtypes
import numpy as np

import concourse.bass as bass  # noqa: F401  (AP helpers)
import concourse.mybir as mybir
import concourse.tile as tile
from concourse.bacc import Bacc
from concourse.bass_utils import run_bass_kernel_spmd
from concourse.masks import make_identity

B = 4
D = 1024
H = 16
DH = 64
N_CORES = 8
HG = 2              # tensor-parallel degree within a batch (head groups)
CL = D // HG        # 512 local channels (8 heads) per core
SCALE = 1.0 / 8.0   # 1 / sqrt(DH)

BF16 = mybir.dt.bfloat16
F32 = mybir.dt.float32
NPBF16 = ml_dtypes.bfloat16
EXP = mybir.ActivationFunctionType.Exp

# Default sequence length; build_nc(T) is parametric for testing.
T_FULL = 2048


def build_nc(T):
    NT = T // 128          # t-tiles
    ND = D // 128          # d-tiles (8)
    NCT = CL // 128        # local c-tiles / head pairs (4)
    NJ = T // 512          # tq chunks
    assert T % 512 == 0

    nc = Bacc(None)
    xT = nc.dram_tensor("xT", [D, T], BF16, kind="ExternalInput")
    wqT = nc.dram_tensor("wqT", [D, CL], BF16, kind="ExternalInput")
    wkT = nc.dram_tensor("wkT", [D, CL], BF16, kind="ExternalInput")
    wvT = nc.dram_tensor("wvT", [D, CL], BF16, kind="ExternalInput")
    woT = nc.dram_tensor("woT", [D, CL], BF16, kind="ExternalInput")
    mask = nc.dram_tensor("mask", [128, 128], BF16, kind="ExternalInput")
    y = nc.dram_tensor("y", [T, CL], F32, kind="ExternalOutput")

    with tile.TileContext(nc) as tc:
        with (
            tc.tile_pool(name="const", bufs=1) as constp,
            tc.tile_pool(name="wo", bufs=1) as wop,
            tc.tile_pool(name="qk", bufs=1) as qkp,
            tc.tile_pool(name="vaug", bufs=1) as vaugp,
            tc.tile_pool(name="outT", bufs=1) as outTp,
            tc.tile_pool(name="dram", bufs=1, space="DRAM") as dramp,
        ):
            mask_sb = constp.tile([128, 128], BF16, tag="mask", name="maskt")
            nc.sync.dma_start(mask_sb[:], mask[:])
            ident = constp.tile([128, 128], BF16, tag="ident", name="ident")
            make_identity(nc, ident[:])

            wo_sb = []
            for ct in range(ND):
                t = wop.tile([128, CL], BF16, tag=f"wo{ct}", name=f"wo{ct}")
                nc.sync.dma_start(t[:], woT[ct * 128:(ct + 1) * 128, :])
                wo_sb.append(t)

            qt_sb = [qkp.tile([128, T], BF16, tag=f"q{ct}", name=f"q{ct}") for ct in range(NCT)]
            kt_sb = [qkp.tile([128, T], BF16, tag=f"k{ct}", name=f"k{ct}") for ct in range(NCT)]
            vaug_sb = [vaugp.tile([128, 8 * 65], BF16, tag=f"v{tt}", name=f"v{tt}") for tt in range(NT)]
            outT_sb = [outTp.tile([128, T], BF16, tag=f"o{ct}", name=f"o{ct}") for ct in range(NCT)]

            ag_in = [dramp.tile([128, T], BF16, tag=f"agi{hp}", name=f"agi{hp}") for hp in range(NCT)]
            ag_out = [dramp.tile([256, T], BF16, tag=f"ago{hp}", name=f"ago{hp}") for hp in range(NCT)]

            # ---------------- QKV projections ----------------
            with (
                tc.tile_pool(name="xtw", bufs=1) as xtwp,
                tc.tile_pool(name="qkvps", bufs=4, space="PSUM") as qkvpsp,
            ):
                xt_sb, wq_sb, wk_sb, wv_sb = [], [], [], []
                for dt in range(ND):
                    t = xtwp.tile([128, T], BF16, tag=f"x{dt}", name=f"x{dt}")
                    nc.sync.dma_start(t[:], xT[dt * 128:(dt + 1) * 128, :])
                    xt_sb.append(t)
                for name, w_dram, lst in (
                    ("wq", wqT, wq_sb), ("wk", wkT, wk_sb), ("wv", wvT, wv_sb)
                ):
                    for dt in range(ND):
                        t = xtwp.tile([128, CL], BF16, tag=f"{name}{dt}", name=f"{name}{dt}")
                        nc.sync.dma_start(t[:], w_dram[dt * 128:(dt + 1) * 128, :])
                        lst.append(t)

                # qT / kT: [c_local, t] = W @ x.T  (W.T tiles stationary)
                for w_sb, dst in ((wq_sb, qt_sb), (wk_sb, kt_sb)):
                    for ct in range(NCT):
                        for tq in range(T // 512):
                            ps = qkvpsp.tile([128, 512], F32, tag="qkvps", name="qkvps")
                            for dt in range(ND):
                                nc.tensor.matmul(
                                    ps[:],
                                    w_sb[dt][:, ct * 128:(ct + 1) * 128],
                                    xt_sb[dt][:, tq * 512:(tq + 1) * 512],
                                    start=(dt == 0), stop=(dt == ND - 1),
                                )
                            nc.vector.tensor_copy(
                                dst[ct][:, tq * 512:(tq + 1) * 512], ps[:]
                            )
                # v: [t, c_local] = x @ W_v.T  (x.T tiles stationary), with a
                # ones column appended per head (softmax denominator).
                for tt in range(NT):
                    ps = qkvpsp.tile([128, 512], F32, tag="qkvps", name="qkvps")
                    for dt in range(ND):
                        nc.tensor.matmul(
                            ps[:],
                            xt_sb[dt][:, tt * 128:(tt + 1) * 128],
                            wv_sb[dt][:],
                            start=(dt == 0), stop=(dt == ND - 1),
                        )
                    nc.vector.memset(vaug_sb[tt][:], 1.0)
                    dst = vaug_sb[tt][:].rearrange("p (h e) -> p h e", e=65)[:, :, 0:64]
                    src = ps[:].rearrange("p (h e) -> p h e", e=64)
                    nc.vector.tensor_copy(dst, src)

            # ---------------- Attention ----------------
            with (
                tc.tile_pool(name="att", bufs=16) as attp,
                tc.tile_pool(name="on", bufs=3) as onp,
                tc.tile_pool(name="rc", bufs=4) as rcp,
                tc.tile_pool(name="stps", bufs=1, space="PSUM") as stpsp,
                tc.tile_pool(name="avps", bufs=3, space="PSUM") as avpsp,
                tc.tile_pool(name="tpps", bufs=1, space="PSUM") as tppsp,
            ):
                def emit_qk_group(hp, J, g, atts):
                    st = stpsp.tile([128, 2048], F32, tag="st", name="st")
                    for u in range(2):
                        i = 2 * g + u
                        for h in range(2):
                            nc.tensor.matmul(
                                st[:, u * 1024 + h * 512:u * 1024 + (h + 1) * 512],
                                kt_sb[hp][h * 64:(h + 1) * 64, i * 128:(i + 1) * 128],
                                qt_sb[hp][h * 64:(h + 1) * 64, J * 512:(J + 1) * 512],
                                start=True, stop=True, tile_position=(h * 64, 0),
                            )
                    att = attp.tile([128, 2048], BF16, tag="att", name="att")
                    nc.scalar.activation(att[:], st[:], EXP, scale=SCALE)
                    for u in range(2):
                        i = 2 * g + u
                        k = i - 4 * J
                        if k >= 0:  # diagonal 128-block: keep tk <= tq
                            for h in range(2):
                                lo = u * 1024 + h * 512 + k * 128
                                nc.vector.tensor_mul(
                                    att[:, lo:lo + 128], att[:, lo:lo + 128], mask_sb[:]
                                )
                    atts.append(att)

                def emit_av_jj(hp, J, jj, atts):
                    jq = 4 * J + jj
                    av = avpsp.tile([128, 130], F32, tag="av", name="av")
                    for h in range(2):
                        for i in range(jq + 1):
                            lhsT = atts[i // 2][
                                :, (i % 2) * 1024 + h * 512 + jj * 128:
                                   (i % 2) * 1024 + h * 512 + (jj + 1) * 128
                            ]
                            hl = hp * 2 + h
                            nc.tensor.matmul(
                                av[:, h * 65:(h + 1) * 65],
                                lhsT,
                                vaug_sb[i][:, hl * 65:(hl + 1) * 65],
                                start=(i == 0), stop=(i == jq),
                            )
                    onorm = onp.tile([128, 128], BF16, tag="on", name="on")
                    for h in range(2):
                        rc = rcp.tile([128, 1], F32, tag="rc", name="rc")
                        nc.vector.reciprocal(rc[:], av[:, h * 65 + 64:h * 65 + 65])
                        nc.vector.tensor_scalar_mul(
                            onorm[:, h * 64:(h + 1) * 64],
                            av[:, h * 65:h * 65 + 64],
                            rc[:],
                        )
                    tp = tppsp.tile([128, 128], BF16, tag="tp", name="tp")
                    nc.tensor.transpose(tp[:], onorm[:], ident[:])
                    nc.vector.tensor_copy(
                        outT_sb[hp][:, J * 512 + jj * 128:J * 512 + (jj + 1) * 128],
                        tp[:],
                    )

                def emit_ag(hp):
                    nc.sync.dma_start(ag_in[hp][:], outT_sb[hp][:])
                    nc.gpsimd.collective_compute(
                        "AllGather",
                        mybir.AluOpType.bypass,
                        replica_groups=[[0, 1], [2, 3], [4, 5], [6, 7]],
                        ins=[ag_in[hp].opt()],
                        outs=[ag_out[hp].opt()],
                    )

                work = [(hp, J) for hp in range(NCT) for J in range(NJ)]
                av_queue = []
                for hp, J in work:
                    n_groups = 2 * J + 2
                    atts = []
                    total_av = len(av_queue)
                    done_av = 0
                    for g in range(n_groups):
                        emit_qk_group(hp, J, g, atts)
                        want = ((g + 1) * total_av) // n_groups
                        while done_av < want:
                            av_queue[done_av]()
                            done_av += 1
                    while done_av < total_av:
                        av_queue[done_av]()
                        done_av += 1
                    av_queue = [
                        (lambda hp=hp, J=J, jj=jj, atts=atts:
                         emit_av_jj(hp, J, jj, atts))
                        for jj in range(4)
                    ]
                    if J == NJ - 1:
                        av_queue.append(lambda hp=hp: emit_ag(hp))
                for c in av_queue:
                    c()

            # ---------------- Output projection ----------------
            with (
                tc.tile_pool(name="ag", bufs=1) as agp,
                tc.tile_pool(name="ysb", bufs=3) as ysbp,
                tc.tile_pool(name="yps", bufs=8, space="PSUM") as ypsp,
            ):
                ag_sb = []
                for ct in range(ND):
                    t = agp.tile([128, T], BF16, tag=f"ag{ct}", name=f"ag{ct}")
                    if ct < NCT:
                        src = ag_out[ct][0:128, :]
                    else:
                        src = ag_out[ct - NCT][128:256, :]
                    nc.sync.dma_start(t[:], src)
                    ag_sb.append(t)
                n_half = NT // 2
                for half in range(2):
                    yps = [ypsp.tile([128, 512], F32, tag="yps", name="yps") for _ in range(n_half)]
                    for ct in range(ND):
                        for tti in range(n_half):
                            tt = half * n_half + tti
                            nc.tensor.matmul(
                                yps[tti][:],
                                ag_sb[ct][:, tt * 128:(tt + 1) * 128],
                                wo_sb[ct][:],
                                start=(ct == 0), stop=(ct == ND - 1),
                            )
                    for tti in range(n_half):
                        tt = half * n_half + tti
                        ysb = ysbp.tile([128, 512], F32, tag="ysb", name="ysb")
                        nc.vector.tensor_copy(ysb[:], yps[tti][:])
                        nc.sync.dma_start(y[tt * 128:(tt + 1) * 128, :], ysb[:])

    nc.compile()
    return nc


_NC_CACHE = {}


def _get_nc(T):
    if T not in _NC_CACHE:
        _NC_CACHE[T] = build_nc(T)
    return _NC_CACHE[T]


def shard_inputs(x, W_q, W_k, W_v, W_o):
    """Host-side sharding: per-core input dicts (bf16, transposed)."""
    T = x.shape[1]
    tri = np.triu(np.ones((128, 128), np.float32)).astype(NPBF16)
    in_maps = []
    for c in range(N_CORES):
        b, hg = c // 2, c % 2
        cs = slice(hg * CL, (hg + 1) * CL)
        in_maps.append({
            "xT": np.ascontiguousarray(x[b].T).astype(NPBF16),
            "wqT": np.ascontiguousarray(W_q[cs, :].T).astype(NPBF16),
            "wkT": np.ascontiguousarray(W_k[cs, :].T).astype(NPBF16),
            "wvT": np.ascontiguousarray(W_v[cs, :].T).astype(NPBF16),
            "woT": np.ascontiguousarray(W_o[cs, :].T).astype(NPBF16),
            "mask": tri,
        })
    return in_maps


def assemble_output(results, T):
    y = np.zeros((B, T, D), np.float32)
    for c in range(N_CORES):
        b, hg = c // 2, c % 2
        y[b][:, hg * CL:(hg + 1) * CL] = results[c]["y"]
    return y


def kernel(x, W_q, W_k, W_v, W_o, _trace=False):
    x = np.asarray(x, dtype=np.float32)
    W_q = np.asarray(W_q, dtype=np.float32)
    W_k = np.asarray(W_k, dtype=np.float32)
    W_v = np.asarray(W_v, dtype=np.float32)
    W_o = np.asarray(W_o, dtype=np.float32)
    T = x.shape[1]
    nc = _get_nc(T)
    in_maps = shard_inputs(x, W_q, W_k, W_v, W_o)
    res = run_bass_kernel_spmd(
        nc, in_maps, core_ids=list(range(N_CORES)), trace=_trace
    )
    out = assemble_output(res.results, T)
    if _trace:
        return out, res
    return out
